# revision 4
# baseline (speedup 1.0000x reference)
"""BitLinear (ternary-weight linear with int8 activation quantization) on 8 trn2 cores.

y = (clip(round(x/x_scale),-128,127) * x_scale) @ (clip(round(w/w_scale),-1,1) * w_scale).T
  x_scale = max(max|x|, eps)/127   (per-tensor)
  w_scale = max(mean|w|, eps)      (per-tensor)

Sharding: tensor-parallel over out_features (11008 = 8 x 1376), x replicated.
The two per-tensor scalar scales are computed host-side (they replicate
trivially); a single device launch quantizes x/w on the fly (exact-integer
bf16 magic rounding) and runs the int8xternary matmul at the bf16 PE roofline.

Prologue: while the weight shard streams in + quantizes (o-tile at a time),
the PE runs the first 512 tokens with SWAPPED operands (wq stationary, xq
moving) so matmuls start as soon as the first 128 weight columns are ready
instead of waiting for a full 512-column slice. Steady state then switches to
x-stationary with 512/512/352 column slices (no partition-granularity waste).
"""

import numpy as np
from contextlib import ExitStack

import concourse.bass as bass
import concourse.tile as tile
from concourse import bacc, mybir
from concourse.bass_utils import run_bass_kernel_spmd

# problem shapes (hardcoded per contract)
B, T, I, O = 4, 2048, 4096, 11008
TOK = B * T                  # 8192
N_CORES = 8
O_SH = O // N_CORES          # 1376
EPS = 1e-5
MAGIC = 12582912.0           # 1.5 * 2**23: fp32 add forces round-to-nearest-even int
F32 = mybir.dt.float32
BF16 = mybir.dt.bfloat16

# tiling
TB = 256                     # tokens per streaming block (2 PSUM m-tiles)
NBLK = TOK // TB             # 32
KT = I // 128                # 32 k-tiles
CH = 8                       # k-tiles per x DMA chunk (CH*TB*4B*128 = 1MB)
NCH = KT // CH               # 4 chunks per block
WKC = 8                      # k-tiles per w quant chunk
OB = (512, 512, 352)         # out-feature split per PSUM bank (sum = 1376)
OB_OFF = (0, 512, 1024)
# prologue (phase S): swapped-operand o-tiles, 2 token sub-windows of 256
OT_W = [128] * 10 + [96]     # 10*128 + 96 = 1376
OT_OFF = [128 * i for i in range(11)]
SBLK = 2                     # x blocks consumed by phase S (tokens 0:512)


def _build():
    nc = bacc.Bacc("TRN2", target_bir_lowering=False, debug=False,
                   num_devices=N_CORES)
    xT = nc.dram_tensor("xT", [I, TOK], F32, kind="ExternalInput").ap()
    wT = nc.dram_tensor("wT", [I, O_SH], F32, kind="ExternalInput").ap()
    consts = nc.dram_tensor("consts", [1, 8], F32, kind="ExternalInput").ap()
    out = nc.dram_tensor("out", [TOK, O_SH], F32, kind="ExternalOutput").ap()
    outS = nc.dram_tensor("outS", [O_SH, TB * SBLK], F32, kind="ExternalOutput").ap()

    xTr = xT.rearrange("(kt p) t -> p kt t", p=128)   # [128, KT, TOK]
    wTr = wT.rearrange("(kt p) o -> p kt o", p=128)   # [128, KT, O_SH]

    with tile.TileContext(nc) as tc:
        with ExitStack() as ctx:
            const_pool = ctx.enter_context(tc.tile_pool(name="const", bufs=1))
            wq_pool = ctx.enter_context(tc.tile_pool(name="wq", bufs=1))
            stage = ctx.enter_context(tc.tile_pool(name="stage", bufs=2))
            rnd = ctx.enter_context(tc.tile_pool(name="rnd", bufs=2))
            wstage = ctx.enter_context(tc.tile_pool(name="wstage", bufs=2))
            wrnd = ctx.enter_context(tc.tile_pool(name="wrnd", bufs=2))
            xq_pool = ctx.enter_context(tc.tile_pool(name="xq", bufs=4))
            out_pool = ctx.enter_context(tc.tile_pool(name="out", bufs=4))
            psum = ctx.enter_context(tc.tile_pool(name="psum", bufs=6, space="PSUM"))

            sb_c = const_pool.tile([128, 8], F32)
            nc.sync.dma_start(sb_c[:], consts.to_broadcast((128, 8)))
            inv_w = sb_c[:, 0:1]
            inv_x = sb_c[:, 1:2]
            out_scale = sb_c[:, 2:3]

            # SBUF-resident ternarized weight shard, bf16 [128, KT, O_SH]
            wq = wq_pool.tile([128, KT, O_SH], BF16)

            def quant_w_otile(ot):
                """quantize w columns [OT_OFF[ot], +OT_W[ot]) over all k."""
                o0, ow = OT_OFF[ot], OT_W[ot]
                for c in range(KT // WKC):
                    wf = wstage.tile([128, WKC, ow], F32, tag="wstage",
                                     name=f"wf{ot}_{c}")
                    nc.sync.dma_start(wf[:], wTr[:, c * WKC:(c + 1) * WKC,
                                              o0:o0 + ow])
                    wr_ = wrnd.tile([128, WKC, ow], F32, tag="wrnd",
                                    name=f"wr{ot}_{c}")
                    # round(w * inv_w) in magic space (ACT: out = in*scale + bias)
                    nc.scalar.activation(wr_[:], wf[:],
                                         mybir.ActivationFunctionType.Copy,
                                         bias=MAGIC, scale=inv_w)
                    # clip to [-1, 1] in magic space (DVE)
                    nc.vector.tensor_scalar(wr_[:], wr_[:], MAGIC + 1.0, MAGIC - 1.0,
                                            op0=mybir.AluOpType.min,
                                            op1=mybir.AluOpType.max)
                    # subtract magic + cast bf16 (ACT, keeps DVE off critical path)
                    nc.scalar.activation(
                        wq[:, c * WKC:(c + 1) * WKC, o0:o0 + ow],
                        wr_[:], mybir.ActivationFunctionType.Copy,
                        bias=-MAGIC)

            xq_tiles = {}

            def quant_x_block(tb):
                t0 = tb * TB
                xq = xq_pool.tile([128, KT, TB], BF16, tag="xq", name=f"xq{tb}")
                xq_tiles[tb] = xq
                for c in range(NCH):
                    xf = stage.tile([128, CH, TB], F32, tag="stage",
                                    name=f"xf{tb}_{c}")
                    nc.sync.dma_start(xf[:], xTr[:, c * CH:(c + 1) * CH,
                                              t0:t0 + TB])
                    xr_ = rnd.tile([128, CH, TB], F32, tag="rnd",
                                   name=f"xr{tb}_{c}")
                    nc.scalar.activation(xr_[:], xf[:],
                                         mybir.ActivationFunctionType.Copy,
                                         bias=MAGIC, scale=inv_x)
                    # no clip needed: |x|/x_scale <= 127 by construction
                    nc.vector.tensor_scalar(
                        xq[:, c * CH:(c + 1) * CH, :],
                        xr_[:], -MAGIC, None, op0=mybir.AluOpType.add)

            def mm_swapped(ot, s):
                """phase S: out[o-tile ot, tokens s*256:(s+1)*256], wq stationary."""
                o0, ow = OT_OFF[ot], OT_W[ot]
                xq = xq_tiles[s]
                ps = psum.tile([128, 512], F32, tag="ps", name=f"psS{ot}_{s}")
                for k in range(KT):
                    nc.tensor.matmul(ps[:ow, :TB],
                                     wq[:, k, o0:o0 + ow],
                                     xq[:, k, :],
                                     start=(k == 0), stop=(k == KT - 1))
                ob = out_pool.tile([128, 512], F32, tag="ob", name=f"obS{ot}_{s}")
                nc.scalar.mul(ob[:ow, :TB], ps[:ow, :TB], out_scale[:ow, :])
                nc.sync.dma_start(outS[o0:o0 + ow, s * TB:(s + 1) * TB],
                                  ob[:ow, :TB])

            def mm_j(tb, j, bs):
                """steady state: matmul for m-tile j of block tb, slices bs."""
                xq = xq_tiles[tb]
                ps = {}
                for b in bs:
                    ps[b] = psum.tile([128, 512], F32, tag="ps",
                                      name=f"ps{tb}_{j}_{b}")
                    for k in range(KT):
                        nc.tensor.matmul(ps[b][:, :OB[b]],
                                         xq[:, k, j * 128:(j + 1) * 128],
                                         wq[:, k, OB_OFF[b]:OB_OFF[b] + OB[b]],
                                         start=(k == 0), stop=(k == KT - 1))
                t0 = tb * TB + j * 128
                for b in bs:
                    ob = out_pool.tile([128, 512], F32, tag="ob",
                                       name=f"ob{tb}_{j}_{b}")
                    nc.scalar.mul(ob[:, :OB[b]], ps[b][:, :OB[b]], out_scale)
                    nc.sync.dma_start(
                        out[t0:t0 + 128, OB_OFF[b]:OB_OFF[b] + OB[b]],
                        ob[:, :OB[b]])

            # --- phase S: swapped-operand prologue over tokens 0:512 ---
            quant_w_otile(0)
            quant_x_block(0)
            quant_w_otile(1)
            quant_x_block(1)
            for ot in range(len(OT_W)):
                if ot + 2 < len(OT_W):
                    quant_w_otile(ot + 2)
                for s in range(SBLK):
                    mm_swapped(ot, s)
            # --- phase C: steady state, x stationary, tokens 512:8192 ---
            for tb in range(SBLK, NBLK):
                quant_x_block(tb)
                for j in range(TB // 128):
                    mm_j(tb, j, [0, 1, 2])
    nc.compile()
    return nc


_cache = {}


def _get_nc():
    if "B" not in _cache:
        _cache["B"] = _build()
    return _cache["B"]


def _run(nc, in_maps, core_ids):
    try:
        return run_bass_kernel_spmd(nc, in_maps, core_ids)
    except Exception:
        import time as _t
        _t.sleep(10)  # transient tunnel/device hiccups recover on retry
        return run_bass_kernel_spmd(nc, in_maps, core_ids)


def kernel(x: np.ndarray, weight: np.ndarray) -> np.ndarray:
    ncB = _get_nc()
    core_ids = list(range(N_CORES))

    x = np.asarray(x)
    weight = np.asarray(weight)
    assert x.shape == (B, T, I) and weight.shape == (O, I), (x.shape, weight.shape)
    x_flat = np.ascontiguousarray(x.reshape(TOK, I), dtype=np.float32)
    weight = np.ascontiguousarray(weight, dtype=np.float32)

    # per-tensor scalar scales (host: they replicate trivially across cores)
    absmax = np.float32(np.max(np.abs(x_flat)))
    wmean = np.float32(np.float32(np.sum(np.abs(weight), dtype=np.float64)) /
                       np.float32(O * I))
    x_scale = np.float32(max(absmax, np.float32(EPS))) / np.float32(127.0)
    w_scale = np.float32(max(wmean, np.float32(EPS)))
    consts = np.zeros((1, 8), dtype=np.float32)
    consts[0, 0] = np.float32(1.0) / w_scale
    consts[0, 1] = np.float32(1.0) / x_scale
    consts[0, 2] = x_scale * w_scale

    # single launch: quantize + exact-integer bf16 matmul, TP over out_features
    xT = np.ascontiguousarray(x_flat.T)               # [I, TOK]
    wTf = weight.T                                    # [I, O] view
    in_B = [{
        "xT": xT,
        "wT": np.ascontiguousarray(wTf[:, i * O_SH:(i + 1) * O_SH]),
        "consts": consts,
    } for i in range(N_CORES)]
    resB = _run(ncB, in_B, core_ids)
    TS = TB * SBLK
    shards = []
    for i in range(N_CORES):
        top = np.ascontiguousarray(resB.results[i]["outS"].T)   # [TS, O_SH]
        rest = resB.results[i]["out"][TS:]                      # [TOK-TS, O_SH]
        shards.append(np.concatenate([top, rest], axis=0))
    out = np.concatenate(shards, axis=1)
    return out.reshape(B, T, O)


# revision 6
# speedup vs baseline: 1.1946x; 1.1946x over previous
"""BitLinear (ternary-weight linear with int8 activation quantization) on 8 trn2 cores.

y = (clip(round(x/x_scale),-128,127) * x_scale) @ (clip(round(w/w_scale),-1,1) * w_scale).T
  x_scale = max(max|x|, eps)/127   (per-tensor)
  w_scale = max(mean|w|, eps)      (per-tensor)

Sharding: tensor-parallel over out_features (11008 = 8 x 1376), x replicated.
The two per-tensor scalar scales are computed host-side (they replicate
trivially); a single device launch quantizes x/w on the fly (exact-integer
bf16 magic rounding) and runs the int8xternary matmul at the bf16 PE roofline.

Prologue: slice-0 weight k-chunks stream into 8 open PSUM accumulation groups
(m-tiles of blocks 0-3) as soon as each chunk is quantized, so the PE starts
~11us in instead of waiting ~60us for the full 512-column slice; m-tiles join
the stream when their xq block is ready and missed k-tiles are caught up once
slice 0 is fully resident.
"""

import numpy as np
from contextlib import ExitStack

import concourse.bass as bass
import concourse.tile as tile
from concourse import bacc, mybir
from concourse.bass_utils import run_bass_kernel_spmd

# problem shapes (hardcoded per contract)
B, T, I, O = 4, 2048, 4096, 11008
TOK = B * T                  # 8192
N_CORES = 8
O_SH = O // N_CORES          # 1376
EPS = 1e-5
MAGIC = 12582912.0           # 1.5 * 2**23: fp32 add forces round-to-nearest-even int
F32 = mybir.dt.float32
BF16 = mybir.dt.bfloat16

# tiling
TB = 256                     # tokens per streaming block (2 PSUM m-tiles)
NBLK = TOK // TB             # 32
KT = I // 128                # 32 k-tiles
CH = 8                       # k-tiles per x DMA chunk (CH*TB*4B*128 = 1MB)
NCH = KT // CH               # 4 chunks per block
WCH = 2                      # k-tiles per w chunk
NWCH = KT // WCH             # 16 chunks per slice
OB = (512, 512, 352)         # out-feature split per PSUM bank (sum = 1376)
OB_OFF = (0, 512, 1024)
EARLY = 4                    # blocks whose slices are emitted per-slice up front
# prologue stream: chunk index at which each EARLY block's m-tiles join
JOIN = (0, 4, 8, 12)


def _build():
    nc = bacc.Bacc("TRN2", target_bir_lowering=False, debug=False,
                   num_devices=N_CORES)
    xT = nc.dram_tensor("xT", [I, TOK], F32, kind="ExternalInput").ap()
    wT = nc.dram_tensor("wT", [I, O_SH], F32, kind="ExternalInput").ap()
    consts = nc.dram_tensor("consts", [1, 8], F32, kind="ExternalInput").ap()
    out = nc.dram_tensor("out", [TOK, O_SH], F32, kind="ExternalOutput").ap()

    xTr = xT.rearrange("(kt p) t -> p kt t", p=128)   # [128, KT, TOK]
    wTr = wT.rearrange("(kt p) o -> p kt o", p=128)   # [128, KT, O_SH]

    with tile.TileContext(nc) as tc:
        with ExitStack() as ctx:
            const_pool = ctx.enter_context(tc.tile_pool(name="const", bufs=1))
            wq_pool = ctx.enter_context(tc.tile_pool(name="wq", bufs=1))
            stage = ctx.enter_context(tc.tile_pool(name="stage", bufs=2))
            rnd = ctx.enter_context(tc.tile_pool(name="rnd", bufs=2))
            wstage = ctx.enter_context(tc.tile_pool(name="wstage", bufs=2))
            wrnd = ctx.enter_context(tc.tile_pool(name="wrnd", bufs=2))
            xq_pool = ctx.enter_context(tc.tile_pool(name="xq", bufs=4))
            out_pool = ctx.enter_context(tc.tile_pool(name="out", bufs=4))
            psum = ctx.enter_context(tc.tile_pool(name="psum", bufs=8, space="PSUM"))

            sb_c = const_pool.tile([128, 8], F32)
            nc.sync.dma_start(sb_c[:], consts.to_broadcast((128, 8)))
            inv_w = sb_c[:, 0:1]
            inv_x = sb_c[:, 1:2]
            out_scale = sb_c[:, 2:3]

            # SBUF-resident ternarized weight shard, bf16 [128, KT, O_SH]
            wq = wq_pool.tile([128, KT, O_SH], BF16)

            def quant_w_chunk(b, c):
                """quantize w chunk c (WCH k-tiles) of column slice b."""
                o0, ow = OB_OFF[b], OB[b]
                wf = wstage.tile([128, WCH, ow], F32, tag="wstage",
                                 name=f"wf{b}_{c}")
                nc.sync.dma_start(wf[:], wTr[:, c * WCH:(c + 1) * WCH,
                                          o0:o0 + ow])
                wr_ = wrnd.tile([128, WCH, ow], F32, tag="wrnd",
                                name=f"wr{b}_{c}")
                # round(w * inv_w) in magic space (ACT: out = in*scale + bias)
                nc.scalar.activation(wr_[:], wf[:],
                                     mybir.ActivationFunctionType.Copy,
                                     bias=MAGIC, scale=inv_w)
                # clip to [-1, 1] in magic space, subtract magic, cast bf16
                nc.vector.tensor_scalar(wr_[:], wr_[:], MAGIC + 1.0, MAGIC - 1.0,
                                        op0=mybir.AluOpType.min,
                                        op1=mybir.AluOpType.max)
                nc.vector.tensor_scalar(
                    wq[:, c * WCH:(c + 1) * WCH, o0:o0 + ow],
                    wr_[:], -MAGIC, None, op0=mybir.AluOpType.add)

            xq_tiles = {}

            def quant_x_block(tb):
                t0 = tb * TB
                xq = xq_pool.tile([128, KT, TB], BF16, tag="xq", name=f"xq{tb}")
                xq_tiles[tb] = xq
                for c in range(NCH):
                    xf = stage.tile([128, CH, TB], F32, tag="stage",
                                    name=f"xf{tb}_{c}")
                    nc.sync.dma_start(xf[:], xTr[:, c * CH:(c + 1) * CH,
                                              t0:t0 + TB])
                    xr_ = rnd.tile([128, CH, TB], F32, tag="rnd",
                                   name=f"xr{tb}_{c}")
                    nc.scalar.activation(xr_[:], xf[:],
                                         mybir.ActivationFunctionType.Copy,
                                         bias=MAGIC, scale=inv_x)
                    # no clip needed: |x|/x_scale <= 127 by construction
                    nc.vector.tensor_scalar(
                        xq[:, c * CH:(c + 1) * CH, :],
                        xr_[:], -MAGIC, None, op0=mybir.AluOpType.add)

            def mm_k(ps, tb, j, b, k, start, stop):
                nc.tensor.matmul(ps[:, :OB[b]],
                                 xq_tiles[tb][:, k, j * 128:(j + 1) * 128],
                                 wq[:, k, OB_OFF[b]:OB_OFF[b] + OB[b]],
                                 start=start, stop=stop,
                                 skip_group_check=True)

            def drain(ps, tb, j, b):
                t0 = tb * TB + j * 128
                ob = out_pool.tile([128, 512], F32, tag="ob",
                                   name=f"ob{tb}_{j}_{b}")
                nc.scalar.mul(ob[:, :OB[b]], ps[:, :OB[b]], out_scale)
                nc.sync.dma_start(
                    out[t0:t0 + 128, OB_OFF[b]:OB_OFF[b] + OB[b]],
                    ob[:, :OB[b]])

            def mm_j(tb, j, bs):
                """full k-loop matmul groups for m-tile j of block tb."""
                for b in bs:
                    ps = psum.tile([128, 512], F32, tag="ps",
                                   name=f"ps{tb}_{j}_{b}")
                    for k in range(KT):
                        mm_k(ps, tb, j, b, k, k == 0, k == KT - 1)
                    drain(ps, tb, j, b)

            # --- prologue: stream slice-0 w k-chunks into 8 open psum groups ---
            quant_w_chunk(0, 0)
            quant_x_block(0)
            quant_w_chunk(0, 1)
            quant_x_block(1)
            quant_w_chunk(0, 2)
            quant_x_block(2)
            quant_w_chunk(0, 3)
            quant_x_block(3)
            ps0 = {}
            for m in range(2 * EARLY):
                ps0[m] = psum.tile([128, 512], F32, tag="ps", name=f"psP{m}")
            for c in range(NWCH):
                if c >= 4:
                    quant_w_chunk(0, c)
                for m in range(2 * EARLY):
                    tb, j = m // 2, m % 2
                    if JOIN[tb] > c:
                        continue
                    for kk in range(WCH):
                        k = c * WCH + kk
                        # stop only for join-0 tiles (no catch-up needed)
                        last = (JOIN[tb] == 0 and c == NWCH - 1 and kk == WCH - 1)
                        mm_k(ps0[m], tb, j, 0, k,
                             start=(c == JOIN[tb] and kk == 0), stop=last)
            # catch-up: k-tiles missed by late-joining m-tiles close their groups
            for m in range(2 * EARLY):
                tb, j = m // 2, m % 2
                nk = JOIN[tb] * WCH
                for k in range(nk):
                    mm_k(ps0[m], tb, j, 0, k, start=False, stop=(k == nk - 1))
            for m in range(2 * EARLY):
                tb, j = m // 2, m % 2
                drain(ps0[m], tb, j, 0)

            # slices 1 and 2 for the EARLY blocks (k-contiguous, full rate)
            for c in range(NWCH):
                quant_w_chunk(1, c)
            for tb in range(EARLY):
                for j in range(2):
                    mm_j(tb, j, [1])
                if tb < 2:
                    for c in range(NWCH // 2 * tb, NWCH // 2 * (tb + 1)):
                        quant_w_chunk(2, c)
            for tb in range(EARLY):
                for j in range(2):
                    mm_j(tb, j, [2])
            # --- steady state ---
            for tb in range(EARLY, NBLK):
                quant_x_block(tb)
                for j in range(TB // 128):
                    mm_j(tb, j, [0, 1, 2])
    nc.compile()
    return nc


_cache = {}


def _get_nc():
    if "B" not in _cache:
        _cache["B"] = _build()
    return _cache["B"]


def _run(nc, in_maps, core_ids):
    try:
        return run_bass_kernel_spmd(nc, in_maps, core_ids)
    except Exception:
        import time as _t
        _t.sleep(10)  # transient tunnel/device hiccups recover on retry
        return run_bass_kernel_spmd(nc, in_maps, core_ids)


def kernel(x: np.ndarray, weight: np.ndarray) -> np.ndarray:
    ncB = _get_nc()
    core_ids = list(range(N_CORES))

    x = np.asarray(x)
    weight = np.asarray(weight)
    assert x.shape == (B, T, I) and weight.shape == (O, I), (x.shape, weight.shape)
    x_flat = np.ascontiguousarray(x.reshape(TOK, I), dtype=np.float32)
    weight = np.ascontiguousarray(weight, dtype=np.float32)

    # per-tensor scalar scales (host: they replicate trivially across cores)
    absmax = np.float32(np.max(np.abs(x_flat)))
    wmean = np.float32(np.float32(np.sum(np.abs(weight), dtype=np.float64)) /
                       np.float32(O * I))
    x_scale = np.float32(max(absmax, np.float32(EPS))) / np.float32(127.0)
    w_scale = np.float32(max(wmean, np.float32(EPS)))
    consts = np.zeros((1, 8), dtype=np.float32)
    consts[0, 0] = np.float32(1.0) / w_scale
    consts[0, 1] = np.float32(1.0) / x_scale
    consts[0, 2] = x_scale * w_scale

    # single launch: quantize + exact-integer bf16 matmul, TP over out_features
    xT = np.ascontiguousarray(x_flat.T)               # [I, TOK]
    wTf = weight.T                                    # [I, O] view
    in_B = [{
        "xT": xT,
        "wT": np.ascontiguousarray(wTf[:, i * O_SH:(i + 1) * O_SH]),
        "consts": consts,
    } for i in range(N_CORES)]
    resB = _run(ncB, in_B, core_ids)
    out = np.concatenate([resB.results[i]["out"] for i in range(N_CORES)], axis=1)
    return out.reshape(B, T, O)
